# revision 14
# baseline (speedup 1.0000x reference)
"""MinGRU fused kernel for Trainium2 (8 NeuronCores).

Problem: x:[4,8192,1024] f32, W:[1024,2048] f32, input_ids:[4,8192] int.
  hg = x @ W; hidden, gate = split(hg)
  linear-domain recurrence h_t = coeff_t * h_{t-1} + value_t with
    coeff = sigmoid(-gate), value = sigmoid(gate) * g(hidden),
    g(x) = max(x + 0.5, sigmoid(x))   (== where(x>=0, x+0.5, sigmoid(x)))
  reset positions (input_ids[s-1]==0): coeff=value=0, done by adding
    +BIG to gate and -BIG to hidden via a K=1 matmul before the sigmoids.
  out = h;  next_prev_hidden = where(ids[:,-1]==0, 0, h[:, -1]).

Sharding: 8 cores = batch(4) x channel-half(2). Each core:
  x[b] cast to fp16 on host, DMA-transposed on device to xT[d, s] tiles;
  W half [1024, 1024] fp16 as stationary; matmul accumulates fp32 in PSUM
  in [e, s] layout; sigmoids on ScalarE; g/value/scan on VectorE
  (tensor_tensor_scan = the recurrence); h transposed back to [s, e] via
  TensorE transpose (PSUM), negation folded into the PSUM->SBUF copy
  (value is computed as -value = (coeff-1)*g so the scan yields -h, and
  the transpose datapath ignores identity values so the sign is restored
  in the ACT copy with scale=-1).
"""
import numpy as np

import concourse.bacc as bacc
import concourse.mybir as mybir
import concourse.tile as tile
from concourse import bass_utils

f32 = mybir.dt.float32
f16 = mybir.dt.float16
AF = mybir.ActivationFunctionType
ALU = mybir.AluOpType

B, S, D = 4, 8192, 1024
EH = D // 2          # channels per core (one half) = 512
SC = 512             # s-chunk (psum free dim)
NCHUNK = S // SC     # 16
TR = 1024            # rows per DMA-transpose issue
BIG = 30000.0        # fp16-representable saturating offset

_PROGRAM = None


def _build_program():
    nc = bacc.Bacc("TRN2", target_bir_lowering=False, debug=False, num_devices=8)

    d_xh = nc.dram_tensor("xh", [S, D], f16, kind="ExternalInput").ap()
    d_wh = nc.dram_tensor("wh", [D, D], f16, kind="ExternalInput").ap()
    d_rst = nc.dram_tensor("rst", [1, S], f16, kind="ExternalInput").ap()
    d_id = nc.dram_tensor("ident", [128, 128], f32, kind="ExternalInput").ap()
    d_out = nc.dram_tensor("out_dram", [EH, S], f32, kind="ExternalOutput").ap()

    with tile.TileContext(nc) as tc:
        with (
            tc.tile_pool(name="wpool", bufs=1) as wpool,
            tc.tile_pool(name="cpool", bufs=1) as cpool,
            tc.tile_pool(name="xtp", bufs=4) as xtp,
            tc.tile_pool(name="cvp", bufs=2) as cvp,
            tc.tile_pool(name="hp", bufs=3) as hp,
            tc.tile_pool(name="hgps", bufs=8, space="PSUM") as hgps,
        ):
            # ---- resident constants / weights ----
            wh_sb = wpool.tile([128, 8 * D], f16, name="wh_sb")
            for j in range(8):
                nc.scalar.dma_start(wh_sb[:, j * D:(j + 1) * D], d_wh[j * 128:(j + 1) * 128, :])
            id_sb = cpool.tile([128, 128], f32, name="id_sb")
            nc.scalar.dma_start(id_sb[:], d_id)
            rst_sb = cpool.tile([1, S], f16, name="rst_sb")
            nc.scalar.dma_start(rst_sb[:], d_rst)
            bigH = cpool.tile([1, 128], f16, name="bigH")
            nc.vector.memset(bigH[:], -BIG)
            bigG = cpool.tile([1, 128], f16, name="bigG")
            nc.vector.memset(bigG[:], BIG)

            def emit_output(k, h_list):
                # store -h shards channel-major; host unshard does the
                # transpose back to [s, e] and the sign flip in one pass
                for p4 in range(4):
                    nc.sync.dma_start(
                        d_out[p4 * 128:(p4 + 1) * 128, k * SC:(k + 1) * SC],
                        h_list[p4][:],
                    )

            def emit_transposes(g2):
                tiles = []
                for j in range(8):
                    t = xtp.tile([128, TR], f16, name=f"xt{j}", tag=f"xt{j}")
                    nc.sync.dma_start(
                        t[:], d_xh[g2 * TR:(g2 + 1) * TR, j * 128:(j + 1) * 128],
                        transpose=True,
                    )
                    tiles.append(t)
                return tiles

            def emit_transposes_split(g2):
                # first group: half-granularity so chunk 0 starts ~40us sooner
                tiles = []
                for j in range(8):
                    t = xtp.tile([128, TR], f16, name=f"xt{j}", tag=f"xt{j}")
                    nc.sync.dma_start(
                        t[:, 0:SC], d_xh[g2 * TR:g2 * TR + SC, j * 128:(j + 1) * 128],
                        transpose=True,
                    )
                    tiles.append(t)
                for j in range(8):
                    nc.sync.dma_start(
                        tiles[j][:, SC:TR],
                        d_xh[g2 * TR + SC:(g2 + 1) * TR, j * 128:(j + 1) * 128],
                        transpose=True,
                    )
                return tiles

            h_prev = [None] * 4
            xts = {0: emit_transposes_split(0), 1: emit_transposes(1)}
            for k in range(NCHUNK):
                g2, half = divmod(k, 2)
                if half == 0:
                    # prefetch two 1024-row groups ahead
                    if g2 + 2 < NCHUNK // 2:
                        xts[g2 + 2] = emit_transposes(g2 + 2)
                    cur_xt = xts[g2]

                # ---- matmuls: 8 e-tiles (0-3 hidden, 4-7 gate) ----
                hg = []
                for i in range(8):
                    p = hgps.tile([128, SC], f32, name=f"hg{i}", tag="hg")
                    for j in range(8):
                        nc.tensor.matmul(
                            p[:],
                            wh_sb[:, j * D + i * 128:j * D + (i + 1) * 128],
                            cur_xt[j][:, half * SC:(half + 1) * SC],
                            start=(j == 0), stop=False,
                        )
                    nc.tensor.matmul(
                        p[:], (bigH if i < 4 else bigG)[:],
                        rst_sb[0:1, k * SC:(k + 1) * SC],
                        start=False, stop=True,
                    )
                    hg.append(p)

                # ---- elementwise + scan per pair ----
                h_cur = []
                for p4 in range(4):
                    hid, gat = hg[p4], hg[p4 + 4]
                    coeff = cvp.tile([128, SC], f32, name=f"coeff{p4}", tag=f"coeff{p4}")
                    nc.scalar.activation(coeff[:], gat[:], AF.Sigmoid, scale=-1.0)
                    sg = cvp.tile([128, SC], f32, name=f"sg{p4}", tag=f"sg{p4}")
                    nc.scalar.activation(sg[:], hid[:], AF.Sigmoid)
                    g = cvp.tile([128, SC], f32, name=f"g{p4}", tag=f"g{p4}")
                    nc.vector.scalar_tensor_tensor(g[:], hid[:], 0.5, sg[:], ALU.add, ALU.max)
                    v = cvp.tile([128, SC], f32, name=f"v{p4}", tag=f"v{p4}")
                    nc.vector.scalar_tensor_tensor(v[:], coeff[:], 1.0, g[:], ALU.subtract, ALU.mult)
                    h = hp.tile([128, SC], f32, name=f"h{p4}", tag=f"h{p4}")
                    init = 0.0 if k == 0 else h_prev[p4][:, SC - 1:SC]
                    nc.vector.tensor_tensor_scan(h[:], coeff[:], v[:], init, ALU.mult, ALU.add)
                    h_cur.append(h)

                emit_output(k, h_cur)
                h_prev = h_cur

    nc.compile()
    return nc


def _get_program():
    global _PROGRAM
    if _PROGRAM is None:
        _PROGRAM = _build_program()
    return _PROGRAM


def _prep_inputs(x, W, input_ids):
    x = np.asarray(x, dtype=np.float32)
    W = np.asarray(W, dtype=np.float32)
    ids = np.asarray(input_ids)
    ident = np.eye(128, dtype=np.float32)
    in_maps = []
    for c in range(8):
        b, half = divmod(c, 2)
        xh = np.ascontiguousarray(x[b]).astype(np.float16)
        wh = np.concatenate(
            [W[:, half * EH:(half + 1) * EH], W[:, D + half * EH:D + (half + 1) * EH]],
            axis=1,
        ).astype(np.float16)
        rst = np.zeros((1, S), dtype=np.float16)
        rst[0, 1:] = (ids[b, :-1] == 0).astype(np.float16)
        in_maps.append({"xh": xh, "wh": wh, "rst": rst, "ident": ident})
    return in_maps


def _assemble(results, input_ids):
    ids = np.asarray(input_ids)
    out = np.empty((B, S, D), dtype=np.float32)
    for c in range(8):
        b, half = divmod(c, 2)
        np.multiply(results[c]["out_dram"].T, -1.0,
                    out=out[b, :, half * EH:(half + 1) * EH])
    nph = np.where((ids[:, -1:] == 0)[:, :, None], np.float32(0), out[:, -1:, :])
    return out, nph.astype(np.float32)


def run(x, W, input_ids, trace=False, trace_cores=None):
    nc = _get_program()
    in_maps = _prep_inputs(x, W, input_ids)
    res = bass_utils.run_bass_kernel_spmd(
        nc, in_maps, core_ids=list(range(8)), trace=trace,
        trace_cores=trace_cores,
    )
    out, nph = _assemble(res.results, input_ids)
    return out, nph, res


def kernel(x, W, input_ids):
    out, nph, _ = run(x, W, input_ids, trace=False)
    return out, nph


# revision 15
# speedup vs baseline: 1.0160x; 1.0160x over previous
"""MinGRU fused kernel for Trainium2 (8 NeuronCores).

Problem: x:[4,8192,1024] f32, W:[1024,2048] f32, input_ids:[4,8192] int.
  hg = x @ W; hidden, gate = split(hg)
  linear-domain recurrence h_t = coeff_t * h_{t-1} + value_t with
    coeff = sigmoid(-gate), value = sigmoid(gate) * g(hidden),
    g(x) = max(x + 0.5, sigmoid(x))   (== where(x>=0, x+0.5, sigmoid(x)))
  reset positions (input_ids[s-1]==0): coeff=value=0, done by adding
    +BIG to gate and -BIG to hidden via a K=1 matmul before the sigmoids.
  out = h;  next_prev_hidden = where(ids[:,-1]==0, 0, h[:, -1]).

Sharding: 8 cores = batch(4) x channel-half(2). Each core:
  x[b] cast to fp16 on host, DMA-transposed on device to xT[d, s] tiles;
  W half [1024, 1024] fp16 as stationary; matmul accumulates fp32 in PSUM
  in [e, s] layout; sigmoids on ScalarE; g/value/scan on VectorE
  (tensor_tensor_scan = the recurrence); h transposed back to [s, e] via
  TensorE transpose (PSUM), negation folded into the PSUM->SBUF copy
  (value is computed as -value = (coeff-1)*g so the scan yields -h, and
  the transpose datapath ignores identity values so the sign is restored
  in the ACT copy with scale=-1).
"""
import numpy as np

import concourse.bacc as bacc
import concourse.mybir as mybir
import concourse.tile as tile
from concourse import bass_utils

f32 = mybir.dt.float32
f16 = mybir.dt.float16
AF = mybir.ActivationFunctionType
ALU = mybir.AluOpType

B, S, D = 4, 8192, 1024
EH = D // 2          # channels per core (one half) = 512
SC = 512             # s-chunk (psum free dim)
NCHUNK = S // SC     # 16
TR = 1024            # rows per DMA-transpose issue
BIG = 30000.0        # fp16-representable saturating offset

_PROGRAM = None


def _build_program():
    nc = bacc.Bacc("TRN2", target_bir_lowering=False, debug=False, num_devices=8)

    d_xh = nc.dram_tensor("xh", [S, D], f16, kind="ExternalInput").ap()
    d_wh = nc.dram_tensor("wh", [D, D], f16, kind="ExternalInput").ap()
    d_rst = nc.dram_tensor("rst", [1, S], f16, kind="ExternalInput").ap()
    d_id = nc.dram_tensor("ident", [128, 128], f32, kind="ExternalInput").ap()
    d_out = nc.dram_tensor("out_dram", [EH, S], f32, kind="ExternalOutput").ap()

    with tile.TileContext(nc) as tc:
        with (
            tc.tile_pool(name="wpool", bufs=1) as wpool,
            tc.tile_pool(name="cpool", bufs=1) as cpool,
            tc.tile_pool(name="xtp", bufs=4) as xtp,
            tc.tile_pool(name="cvp", bufs=2) as cvp,
            tc.tile_pool(name="hp", bufs=3) as hp,
            tc.tile_pool(name="hgps", bufs=8, space="PSUM") as hgps,
        ):
            # ---- resident constants / weights ----
            wh_sb = wpool.tile([128, 8 * D], f16, name="wh_sb")
            for j in range(8):
                nc.scalar.dma_start(wh_sb[:, j * D:(j + 1) * D], d_wh[j * 128:(j + 1) * 128, :])
            id_sb = cpool.tile([128, 128], f32, name="id_sb")
            nc.scalar.dma_start(id_sb[:], d_id)
            rst_sb = cpool.tile([1, S], f16, name="rst_sb")
            nc.scalar.dma_start(rst_sb[:], d_rst)
            bigH = cpool.tile([1, 128], f16, name="bigH")
            nc.vector.memset(bigH[:], -BIG)
            bigG = cpool.tile([1, 128], f16, name="bigG")
            nc.vector.memset(bigG[:], BIG)

            def emit_output(k, h_list):
                # store -h shards channel-major; host unshard does the
                # transpose back to [s, e] and the sign flip in one pass
                for p4 in range(4):
                    nc.sync.dma_start(
                        d_out[p4 * 128:(p4 + 1) * 128, k * SC:(k + 1) * SC],
                        h_list[p4][:],
                    )

            def emit_transposes(g2):
                tiles = []
                for j in range(8):
                    t = xtp.tile([128, TR], f16, name=f"xt{j}", tag=f"xt{j}")
                    nc.sync.dma_start(
                        t[:], d_xh[g2 * TR:(g2 + 1) * TR, j * 128:(j + 1) * 128],
                        transpose=True,
                    )
                    tiles.append(t)
                return tiles

            h_prev = [None] * 4
            xts = {0: emit_transposes(0), 1: emit_transposes(1)}
            for k in range(NCHUNK):
                g2, half = divmod(k, 2)
                if half == 0:
                    # prefetch two 1024-row groups ahead
                    if g2 + 2 < NCHUNK // 2:
                        xts[g2 + 2] = emit_transposes(g2 + 2)
                    cur_xt = xts[g2]

                # ---- matmuls: 8 e-tiles (0-3 hidden, 4-7 gate) ----
                hg = []
                for i in range(8):
                    p = hgps.tile([128, SC], f32, name=f"hg{i}", tag="hg")
                    for j in range(8):
                        nc.tensor.matmul(
                            p[:],
                            wh_sb[:, j * D + i * 128:j * D + (i + 1) * 128],
                            cur_xt[j][:, half * SC:(half + 1) * SC],
                            start=(j == 0), stop=False,
                        )
                    nc.tensor.matmul(
                        p[:], (bigH if i < 4 else bigG)[:],
                        rst_sb[0:1, k * SC:(k + 1) * SC],
                        start=False, stop=True,
                    )
                    hg.append(p)

                # ---- elementwise + scan per pair ----
                h_cur = []
                for p4 in range(4):
                    hid, gat = hg[p4], hg[p4 + 4]
                    coeff = cvp.tile([128, SC], f32, name=f"coeff{p4}", tag=f"coeff{p4}")
                    nc.scalar.activation(coeff[:], gat[:], AF.Sigmoid, scale=-1.0)
                    sg = cvp.tile([128, SC], f32, name=f"sg{p4}", tag=f"sg{p4}")
                    nc.scalar.activation(sg[:], hid[:], AF.Sigmoid)
                    g = cvp.tile([128, SC], f32, name=f"g{p4}", tag=f"g{p4}")
                    nc.vector.scalar_tensor_tensor(g[:], hid[:], 0.5, sg[:], ALU.add, ALU.max)
                    v = cvp.tile([128, SC], f32, name=f"v{p4}", tag=f"v{p4}")
                    nc.vector.scalar_tensor_tensor(v[:], coeff[:], 1.0, g[:], ALU.subtract, ALU.mult)
                    h = hp.tile([128, SC], f32, name=f"h{p4}", tag=f"h{p4}")
                    init = 0.0 if k == 0 else h_prev[p4][:, SC - 1:SC]
                    nc.vector.tensor_tensor_scan(h[:], coeff[:], v[:], init, ALU.mult, ALU.add)
                    h_cur.append(h)

                emit_output(k, h_cur)
                h_prev = h_cur

    nc.compile()
    return nc


def _get_program():
    global _PROGRAM
    if _PROGRAM is None:
        _PROGRAM = _build_program()
    return _PROGRAM


def _prep_inputs(x, W, input_ids):
    x = np.asarray(x, dtype=np.float32)
    W = np.asarray(W, dtype=np.float32)
    ids = np.asarray(input_ids)
    ident = np.eye(128, dtype=np.float32)
    in_maps = []
    for c in range(8):
        b, half = divmod(c, 2)
        xh = np.ascontiguousarray(x[b]).astype(np.float16)
        wh = np.concatenate(
            [W[:, half * EH:(half + 1) * EH], W[:, D + half * EH:D + (half + 1) * EH]],
            axis=1,
        ).astype(np.float16)
        rst = np.zeros((1, S), dtype=np.float16)
        rst[0, 1:] = (ids[b, :-1] == 0).astype(np.float16)
        in_maps.append({"xh": xh, "wh": wh, "rst": rst, "ident": ident})
    return in_maps


def _assemble(results, input_ids):
    ids = np.asarray(input_ids)
    out = np.empty((B, S, D), dtype=np.float32)
    for c in range(8):
        b, half = divmod(c, 2)
        np.multiply(results[c]["out_dram"].T, -1.0,
                    out=out[b, :, half * EH:(half + 1) * EH])
    nph = np.where((ids[:, -1:] == 0)[:, :, None], np.float32(0), out[:, -1:, :])
    return out, nph.astype(np.float32)


def run(x, W, input_ids, trace=False, trace_cores=None):
    nc = _get_program()
    in_maps = _prep_inputs(x, W, input_ids)
    res = bass_utils.run_bass_kernel_spmd(
        nc, in_maps, core_ids=list(range(8)), trace=trace,
        trace_cores=trace_cores,
    )
    out, nph = _assemble(res.results, input_ids)
    return out, nph, res


def kernel(x, W, input_ids):
    out, nph, _ = run(x, W, input_ids, trace=False)
    return out, nph


# revision 16
# speedup vs baseline: 1.0347x; 1.0185x over previous
"""MinGRU fused kernel for Trainium2 (8 NeuronCores).

Problem: x:[4,8192,1024] f32, W:[1024,2048] f32, input_ids:[4,8192] int.
  hg = x @ W; hidden, gate = split(hg)
  linear-domain recurrence h_t = coeff_t * h_{t-1} + value_t with
    coeff = sigmoid(-gate), value = sigmoid(gate) * g(hidden),
    g(x) = max(x + 0.5, sigmoid(x))   (== where(x>=0, x+0.5, sigmoid(x)))
  reset positions (input_ids[s-1]==0): coeff=value=0, done by adding
    +BIG to gate and -BIG to hidden via a K=1 matmul before the sigmoids.
  out = h;  next_prev_hidden = where(ids[:,-1]==0, 0, h[:, -1]).

Sharding: 8 cores = batch(4) x channel-half(2). Each core:
  x[b] cast to fp16 on host, DMA-transposed on device to xT[d, s] tiles;
  W half [1024, 1024] fp16 as stationary; matmul accumulates fp32 in PSUM
  in [e, s] layout; sigmoids on ScalarE; g/value/scan on VectorE
  (tensor_tensor_scan = the recurrence); h transposed back to [s, e] via
  TensorE transpose (PSUM), negation folded into the PSUM->SBUF copy
  (value is computed as -value = (coeff-1)*g so the scan yields -h, and
  the transpose datapath ignores identity values so the sign is restored
  in the ACT copy with scale=-1).
"""
import numpy as np

import concourse.bacc as bacc
import concourse.mybir as mybir
import concourse.tile as tile
from concourse import bass_utils

f32 = mybir.dt.float32
f16 = mybir.dt.float16
AF = mybir.ActivationFunctionType
ALU = mybir.AluOpType

B, S, D = 4, 8192, 1024
EH = D // 2          # channels per core (one half) = 512
SC = 512             # s-chunk (psum free dim)
NCHUNK = S // SC     # 16
TR = 1024            # rows per DMA-transpose issue
BIG = 30000.0        # fp16-representable saturating offset

_PROGRAM = None


def _build_program():
    nc = bacc.Bacc("TRN2", target_bir_lowering=False, debug=False, num_devices=8)

    d_xh = nc.dram_tensor("xh", [S, D], f16, kind="ExternalInput").ap()
    d_wh = nc.dram_tensor("wh", [D, D], f16, kind="ExternalInput").ap()
    d_rst = nc.dram_tensor("rst", [1, S], f16, kind="ExternalInput").ap()
    d_id = nc.dram_tensor("ident", [128, 128], f32, kind="ExternalInput").ap()
    d_out = nc.dram_tensor("out_dram", [EH, S], f32, kind="ExternalOutput").ap()

    with tile.TileContext(nc) as tc:
        with (
            tc.tile_pool(name="wpool", bufs=1) as wpool,
            tc.tile_pool(name="cpool", bufs=1) as cpool,
            tc.tile_pool(name="xtp", bufs=4) as xtp,
            tc.tile_pool(name="cvp", bufs=2) as cvp,
            tc.tile_pool(name="hp", bufs=3) as hp,
            tc.tile_pool(name="hgps", bufs=8, space="PSUM") as hgps,
        ):
            # ---- resident constants / weights ----
            wh_sb = wpool.tile([128, 8 * D], f16, name="wh_sb")
            for j in range(8):
                nc.scalar.dma_start(wh_sb[:, j * D:(j + 1) * D], d_wh[j * 128:(j + 1) * 128, :])
            id_sb = cpool.tile([128, 128], f32, name="id_sb")
            nc.scalar.dma_start(id_sb[:], d_id)
            rst_sb = cpool.tile([1, S], f16, name="rst_sb")
            nc.scalar.dma_start(rst_sb[:], d_rst)
            bigH = cpool.tile([1, 128], f16, name="bigH")
            nc.vector.memset(bigH[:], -BIG)
            bigG = cpool.tile([1, 128], f16, name="bigG")
            nc.vector.memset(bigG[:], BIG)

            def emit_output(k, h_list):
                # store -h shards channel-major; host unshard does the
                # transpose back to [s, e] and the sign flip in one pass
                for p4 in range(4):
                    nc.sync.dma_start(
                        d_out[p4 * 128:(p4 + 1) * 128, k * SC:(k + 1) * SC],
                        h_list[p4][:],
                    )

            def emit_transposes(g2):
                tiles = []
                for j in range(8):
                    t = xtp.tile([128, TR], f16, name=f"xt{j}", tag=f"xt{j}")
                    nc.sync.dma_start(
                        t[:], d_xh[g2 * TR:(g2 + 1) * TR, j * 128:(j + 1) * 128],
                        transpose=True,
                    )
                    tiles.append(t)
                return tiles

            # pair-wave schedule: per 1024-row group, process e-tile pairs
            # (p4, p4+4) for BOTH chunks with each weight tile used twice in
            # a row (better weight-load overlap); 4 psum banks per wave, two
            # waves in flight.
            h_last = [None] * 4
            xts = {0: emit_transposes(0), 1: emit_transposes(1)}
            for g2 in range(NCHUNK // 2):
                if g2 + 2 < NCHUNK // 2:
                    xts[g2 + 2] = emit_transposes(g2 + 2)
                cur_xt = xts[g2]
                for p4 in range(4):
                    ps = {}
                    for i in (p4, p4 + 4):
                        for kk in (0, 1):
                            ps[(i, kk)] = hgps.tile(
                                [128, SC], f32, name=f"hg{i}_{kk}", tag="hg"
                            )
                    for j in range(8):
                        for i in (p4, p4 + 4):
                            w = wh_sb[:, j * D + i * 128:j * D + (i + 1) * 128]
                            for kk in (0, 1):
                                nc.tensor.matmul(
                                    ps[(i, kk)][:], w,
                                    cur_xt[j][:, kk * SC:(kk + 1) * SC],
                                    start=(j == 0), stop=False,
                                )
                    for i in (p4, p4 + 4):
                        for kk in (0, 1):
                            k = 2 * g2 + kk
                            nc.tensor.matmul(
                                ps[(i, kk)][:], (bigH if i < 4 else bigG)[:],
                                rst_sb[0:1, k * SC:(k + 1) * SC],
                                start=False, stop=True,
                            )
                    for kk in (0, 1):
                        k = 2 * g2 + kk
                        hid, gat = ps[(p4, kk)], ps[(p4 + 4, kk)]
                        coeff = cvp.tile([128, SC], f32, name=f"coeff{p4}", tag=f"coeff{p4}")
                        nc.scalar.activation(coeff[:], gat[:], AF.Sigmoid, scale=-1.0)
                        sg = cvp.tile([128, SC], f32, name=f"sg{p4}", tag=f"sg{p4}")
                        nc.scalar.activation(sg[:], hid[:], AF.Sigmoid)
                        g = cvp.tile([128, SC], f32, name=f"g{p4}", tag=f"g{p4}")
                        nc.vector.scalar_tensor_tensor(g[:], hid[:], 0.5, sg[:], ALU.add, ALU.max)
                        v = cvp.tile([128, SC], f32, name=f"v{p4}", tag=f"v{p4}")
                        nc.vector.scalar_tensor_tensor(v[:], coeff[:], 1.0, g[:], ALU.subtract, ALU.mult)
                        h = hp.tile([128, SC], f32, name=f"h{p4}", tag=f"h{p4}")
                        init = 0.0 if k == 0 else h_last[p4][:, SC - 1:SC]
                        nc.vector.tensor_tensor_scan(h[:], coeff[:], v[:], init, ALU.mult, ALU.add)
                        h_last[p4] = h
                        nc.sync.dma_start(
                            d_out[p4 * 128:(p4 + 1) * 128, k * SC:(k + 1) * SC],
                            h[:],
                        )

    nc.compile()
    return nc


def _get_program():
    global _PROGRAM
    if _PROGRAM is None:
        _PROGRAM = _build_program()
    return _PROGRAM


def _prep_inputs(x, W, input_ids):
    x = np.asarray(x, dtype=np.float32)
    W = np.asarray(W, dtype=np.float32)
    ids = np.asarray(input_ids)
    ident = np.eye(128, dtype=np.float32)
    in_maps = []
    for c in range(8):
        b, half = divmod(c, 2)
        xh = np.ascontiguousarray(x[b]).astype(np.float16)
        wh = np.concatenate(
            [W[:, half * EH:(half + 1) * EH], W[:, D + half * EH:D + (half + 1) * EH]],
            axis=1,
        ).astype(np.float16)
        rst = np.zeros((1, S), dtype=np.float16)
        rst[0, 1:] = (ids[b, :-1] == 0).astype(np.float16)
        in_maps.append({"xh": xh, "wh": wh, "rst": rst, "ident": ident})
    return in_maps


def _assemble(results, input_ids):
    ids = np.asarray(input_ids)
    out = np.empty((B, S, D), dtype=np.float32)
    for c in range(8):
        b, half = divmod(c, 2)
        np.multiply(results[c]["out_dram"].T, -1.0,
                    out=out[b, :, half * EH:(half + 1) * EH])
    nph = np.where((ids[:, -1:] == 0)[:, :, None], np.float32(0), out[:, -1:, :])
    return out, nph.astype(np.float32)


def run(x, W, input_ids, trace=False, trace_cores=None):
    nc = _get_program()
    in_maps = _prep_inputs(x, W, input_ids)
    res = bass_utils.run_bass_kernel_spmd(
        nc, in_maps, core_ids=list(range(8)), trace=trace,
        trace_cores=trace_cores,
    )
    out, nph = _assemble(res.results, input_ids)
    return out, nph, res


def kernel(x, W, input_ids):
    out, nph, _ = run(x, W, input_ids, trace=False)
    return out, nph


# revision 17
# speedup vs baseline: 1.0573x; 1.0218x over previous
"""MinGRU fused kernel for Trainium2 (8 NeuronCores).

Problem: x:[4,8192,1024] f32, W:[1024,2048] f32, input_ids:[4,8192] int.
  hg = x @ W; hidden, gate = split(hg)
  linear-domain recurrence h_t = coeff_t * h_{t-1} + value_t with
    coeff = sigmoid(-gate), value = sigmoid(gate) * g(hidden),
    g(x) = max(x + 0.5, sigmoid(x))   (== where(x>=0, x+0.5, sigmoid(x)))
  reset positions (input_ids[s-1]==0): coeff=value=0, done by adding
    +BIG to gate and -BIG to hidden via a K=1 matmul before the sigmoids.
  out = h;  next_prev_hidden = where(ids[:,-1]==0, 0, h[:, -1]).

Sharding: 8 cores = batch(4) x channel-half(2). Each core:
  x[b] cast to fp16 on host, DMA-transposed on device to xT[d, s] tiles;
  W half [1024, 1024] fp16 as stationary; matmul accumulates fp32 in PSUM
  in [e, s] layout; sigmoids on ScalarE; g/value/scan on VectorE
  (tensor_tensor_scan = the recurrence); h transposed back to [s, e] via
  TensorE transpose (PSUM), negation folded into the PSUM->SBUF copy
  (value is computed as -value = (coeff-1)*g so the scan yields -h, and
  the transpose datapath ignores identity values so the sign is restored
  in the ACT copy with scale=-1).
"""
import numpy as np

import concourse.bacc as bacc
import concourse.mybir as mybir
import concourse.tile as tile
from concourse import bass_utils

f32 = mybir.dt.float32
f16 = mybir.dt.float16
AF = mybir.ActivationFunctionType
ALU = mybir.AluOpType

B, S, D = 4, 8192, 1024
EH = D // 2          # channels per core (one half) = 512
SC = 512             # s-chunk (psum free dim)
NCHUNK = S // SC     # 16
TR = 1024            # rows per DMA-transpose issue
BIG = 30000.0        # fp16-representable saturating offset

_PROGRAM = None


def _build_program():
    nc = bacc.Bacc("TRN2", target_bir_lowering=False, debug=False, num_devices=8)

    d_xh = nc.dram_tensor("xh", [S, D], f16, kind="ExternalInput").ap()
    d_wh = nc.dram_tensor("wh", [D, D], f16, kind="ExternalInput").ap()
    d_rst = nc.dram_tensor("rst", [1, S], f16, kind="ExternalInput").ap()
    d_id = nc.dram_tensor("ident", [128, 128], f32, kind="ExternalInput").ap()
    d_out = nc.dram_tensor("out_dram", [EH, S], f32, kind="ExternalOutput").ap()

    with tile.TileContext(nc) as tc:
        with (
            tc.tile_pool(name="wpool", bufs=1) as wpool,
            tc.tile_pool(name="cpool", bufs=1) as cpool,
            tc.tile_pool(name="xtp", bufs=4) as xtp,
            tc.tile_pool(name="cvp", bufs=2) as cvp,
            tc.tile_pool(name="hp", bufs=3) as hp,
            tc.tile_pool(name="hgps", bufs=8, space="PSUM") as hgps,
        ):
            # ---- resident constants / weights ----
            wh_sb = wpool.tile([128, 8 * D], f16, name="wh_sb")
            for j in range(8):
                nc.sync.dma_start(wh_sb[:, j * D:(j + 1) * D], d_wh[j * 128:(j + 1) * 128, :])
            id_sb = cpool.tile([128, 128], f32, name="id_sb")
            nc.sync.dma_start(id_sb[:], d_id)
            rst_sb = cpool.tile([1, S], f16, name="rst_sb")
            nc.sync.dma_start(rst_sb[:], d_rst)
            bigH = cpool.tile([1, 128], f16, name="bigH")
            nc.vector.memset(bigH[:], -BIG)
            bigG = cpool.tile([1, 128], f16, name="bigG")
            nc.vector.memset(bigG[:], BIG)

            def emit_output(k, h_list):
                # store -h shards channel-major; host unshard does the
                # transpose back to [s, e] and the sign flip in one pass
                for p4 in range(4):
                    nc.sync.dma_start(
                        d_out[p4 * 128:(p4 + 1) * 128, k * SC:(k + 1) * SC],
                        h_list[p4][:],
                    )

            def emit_transposes(g2, eng=None):
                eng = eng or nc.sync
                tiles = []
                for j in range(8):
                    t = xtp.tile([128, TR], f16, name=f"xt{j}", tag=f"xt{j}")
                    eng.dma_start(
                        t[:], d_xh[g2 * TR:(g2 + 1) * TR, j * 128:(j + 1) * 128],
                        transpose=True,
                    )
                    tiles.append(t)
                return tiles

            # pair-wave schedule: per 1024-row group, process e-tile pairs
            # (p4, p4+4) for BOTH chunks with each weight tile used twice in
            # a row (better weight-load overlap); 4 psum banks per wave, two
            # waves in flight.
            h_last = [None] * 4
            xts = {0: emit_transposes(0, nc.scalar), 1: emit_transposes(1)}
            for g2 in range(NCHUNK // 2):
                if g2 + 2 < NCHUNK // 2:
                    xts[g2 + 2] = emit_transposes(g2 + 2)
                cur_xt = xts[g2]
                for p4 in range(4):
                    ps = {}
                    for i in (p4, p4 + 4):
                        for kk in (0, 1):
                            ps[(i, kk)] = hgps.tile(
                                [128, SC], f32, name=f"hg{i}_{kk}", tag="hg"
                            )
                    for j in range(8):
                        for i in (p4, p4 + 4):
                            w = wh_sb[:, j * D + i * 128:j * D + (i + 1) * 128]
                            for kk in (0, 1):
                                nc.tensor.matmul(
                                    ps[(i, kk)][:], w,
                                    cur_xt[j][:, kk * SC:(kk + 1) * SC],
                                    start=(j == 0), stop=False,
                                )
                    for i in (p4, p4 + 4):
                        for kk in (0, 1):
                            k = 2 * g2 + kk
                            nc.tensor.matmul(
                                ps[(i, kk)][:], (bigH if i < 4 else bigG)[:],
                                rst_sb[0:1, k * SC:(k + 1) * SC],
                                start=False, stop=True,
                            )
                    for kk in (0, 1):
                        k = 2 * g2 + kk
                        hid, gat = ps[(p4, kk)], ps[(p4 + 4, kk)]
                        coeff = cvp.tile([128, SC], f32, name=f"coeff{p4}", tag=f"coeff{p4}")
                        nc.scalar.activation(coeff[:], gat[:], AF.Sigmoid, scale=-1.0)
                        sg = cvp.tile([128, SC], f32, name=f"sg{p4}", tag=f"sg{p4}")
                        nc.scalar.activation(sg[:], hid[:], AF.Sigmoid)
                        g = cvp.tile([128, SC], f32, name=f"g{p4}", tag=f"g{p4}")
                        nc.vector.scalar_tensor_tensor(g[:], hid[:], 0.5, sg[:], ALU.add, ALU.max)
                        v = cvp.tile([128, SC], f32, name=f"v{p4}", tag=f"v{p4}")
                        nc.vector.scalar_tensor_tensor(v[:], coeff[:], 1.0, g[:], ALU.subtract, ALU.mult)
                        h = hp.tile([128, SC], f32, name=f"h{p4}", tag=f"h{p4}")
                        init = 0.0 if k == 0 else h_last[p4][:, SC - 1:SC]
                        nc.vector.tensor_tensor_scan(h[:], coeff[:], v[:], init, ALU.mult, ALU.add)
                        h_last[p4] = h
                        nc.sync.dma_start(
                            d_out[p4 * 128:(p4 + 1) * 128, k * SC:(k + 1) * SC],
                            h[:],
                        )

    nc.compile()
    return nc


def _get_program():
    global _PROGRAM
    if _PROGRAM is None:
        _PROGRAM = _build_program()
    return _PROGRAM


def _prep_inputs(x, W, input_ids):
    x = np.asarray(x, dtype=np.float32)
    W = np.asarray(W, dtype=np.float32)
    ids = np.asarray(input_ids)
    ident = np.eye(128, dtype=np.float32)
    in_maps = []
    for c in range(8):
        b, half = divmod(c, 2)
        xh = np.ascontiguousarray(x[b]).astype(np.float16)
        wh = np.concatenate(
            [W[:, half * EH:(half + 1) * EH], W[:, D + half * EH:D + (half + 1) * EH]],
            axis=1,
        ).astype(np.float16)
        rst = np.zeros((1, S), dtype=np.float16)
        rst[0, 1:] = (ids[b, :-1] == 0).astype(np.float16)
        in_maps.append({"xh": xh, "wh": wh, "rst": rst, "ident": ident})
    return in_maps


def _assemble(results, input_ids):
    ids = np.asarray(input_ids)
    out = np.empty((B, S, D), dtype=np.float32)
    for c in range(8):
        b, half = divmod(c, 2)
        np.multiply(results[c]["out_dram"].T, -1.0,
                    out=out[b, :, half * EH:(half + 1) * EH])
    nph = np.where((ids[:, -1:] == 0)[:, :, None], np.float32(0), out[:, -1:, :])
    return out, nph.astype(np.float32)


def run(x, W, input_ids, trace=False, trace_cores=None):
    nc = _get_program()
    in_maps = _prep_inputs(x, W, input_ids)
    res = bass_utils.run_bass_kernel_spmd(
        nc, in_maps, core_ids=list(range(8)), trace=trace,
        trace_cores=trace_cores,
    )
    out, nph = _assemble(res.results, input_ids)
    return out, nph, res


def kernel(x, W, input_ids):
    out, nph, _ = run(x, W, input_ids, trace=False)
    return out, nph
